# revision 4
# baseline (speedup 1.0000x reference)
"""FlowNet-style correlation layer (MAX_DISPLACEMENT=4, 81 channels) on 8 TRN2 cores.

Strategy
--------
Data-parallel over batch N=8 -> 1 sample per NeuronCore.

Per core, for each 8x16 spatial block of data1 (the "stationary" block, M=128
positions) we matmul against the surrounding 16x24 patch of zero-padded data2
(the "moving" block, N=384 columns), contracting over C=256 in two K=128
chunks accumulated in PSUM.  The [128, 384] PSUM tile contains, for every
stationary position m=(g,j), all 81 correlation values at band columns
(g+dyp)*24 + (j+dxp).

DMA-volume optimizations over the naive banded scheme:
  * d2 is staged ONCE in SBUF as a persistent [128, 2, 104, 168] tile
    (x-padding baked on host, y-padding memset on-chip) -> no overlapping
    slab duplication.  d1 is staged raw [128, 2, 96, 160]; matmul operands
    are strided 2D-AP slices of the persistent tiles.
  * the band output is compacted per 16-partition group g to the 216 band
    columns [24g, 24g+216) that group actually uses (81/216 useful vs
    81/384) before leaving the chip.
  * d1 is pre-scaled by 1/C on the host so the PSUM->SBUF drain is a plain
    fp32->fp16 copy.

The final per-(g,j) diagonal gather (81 of 216 columns, per-partition offset)
happens on the host in numpy - per-partition-varying offsets are not
expressible in a single engine/DMA access pattern.
"""

import numpy as np

C, H, W = 256, 96, 160
PAD = 4
NG = 9  # displacement grid width (2*4+1)
Q = NG * NG  # 81 output channels
GB, BB = 8, 16  # stationary block: GB y-rows x BB x-cols = 128 positions
TT, UU = GB + 2 * PAD, BB + 2 * PAD  # moving block: 16 rows x 24 cols
NBY, NBX = H // GB, W // BB  # 12 x 10 = 120 blocks
NMOV = TT * UU  # 384 moving columns per matmul
HP, WPX = H + 2 * PAD, W + 2 * PAD  # padded dims (104, 168)
NGRP = 8  # partition groups of 16 (one per g)
GCOL = NG * UU  # 216 useful band columns per group
N_CORES = 8

_CACHE = {}


def _build_bass(reps=1):
    import contextlib

    import concourse.bass as bass  # noqa: F401
    import concourse.mybir as mybir
    import concourse.tile as tile
    from concourse import bacc

    fp16 = mybir.dt.float16
    fp32 = mybir.dt.float32

    nc = bacc.Bacc("TRN2", target_bir_lowering=False, debug=False)

    # d1: [2k, 128p, NBY, NBX, 128m] host-blocked m=(g,j) (pre-scaled by 1/C)
    # -- the matmul stationary AP must be a single free dim.  d2: [2k, 128p,
    # H, W+2*PAD] (x-padding baked in on host).
    d1 = nc.dram_tensor(
        "d1", [2, 128, NBY, NBX, 128], fp16, kind="ExternalInput"
    ).ap()
    d2 = nc.dram_tensor("d2", [2, 128, H, WPX], fp16, kind="ExternalInput").ap()
    gout = nc.dram_tensor(
        "gout", [NBY, NGRP, BB, NBX, GCOL], fp16, kind="ExternalOutput"
    ).ap()

    d1r = d1.rearrange("k p by bx m -> p k by bx m")
    d2r = d2.rearrange("k p h w -> p k h w")

    with tile.TileContext(nc) as tc:
        with (
            tc.tile_pool(name="in1", bufs=1) as in1_pool,
            tc.tile_pool(name="in2", bufs=1) as in2_pool,
            tc.tile_pool(name="ps", bufs=8, space="PSUM") as ps_pool,
            tc.tile_pool(name="ob", bufs=3) as ob_pool,
        ):
            loop = tc.For_i(0, reps, 1) if reps > 1 else contextlib.nullcontext()
            with loop:
                d1_sb = in1_pool.tile([128, 2, NBY, NBX, 128], fp16, tag="d1sb")
                d2_sb = in2_pool.tile([128, 2, HP, WPX], fp16, tag="d2sb")
                # y-pad rows of the persistent d2 tile.
                nc.vector.memset(d2_sb[:, :, 0:PAD, :], 0.0)
                nc.vector.memset(d2_sb[:, :, H + PAD : HP, :], 0.0)
                # Chunked loads (8 y-rows each) so compute starts early;
                # separate queues so the two input streams overlap.
                for by in range(NBY):
                    y0 = by * GB
                    nc.gpsimd.dma_start(
                        out=d1_sb[:, :, by], in_=d1r[:, :, by]
                    )
                    nc.sync.dma_start(
                        out=d2_sb[:, :, PAD + y0 : PAD + y0 + GB, :],
                        in_=d2r[:, :, y0 : y0 + GB, :],
                    )
                for by in range(NBY):
                    y0 = by * GB
                    ob = ob_pool.tile([128, NBX, NMOV], fp16, tag="ob")
                    for bx in range(NBX):
                        x0 = bx * BB
                        ps = ps_pool.tile([128, NMOV], fp32)
                        for k in range(2):
                            lhsT = d1_sb[:, k, by, bx, :]
                            rhs = d2_sb[:, k, y0 : y0 + TT, x0 : x0 + UU]
                            nc.tensor.matmul(
                                ps[:], lhsT, rhs, start=(k == 0), stop=(k == 1)
                            )
                        if bx % 2 == 1:
                            nc.scalar.copy(ob[:, bx, :], ps[:])
                        else:
                            nc.vector.tensor_copy(ob[:, bx, :], ps[:])
                    # Grouped band compaction: partition group g only uses
                    # band columns [24g, 24g+216).
                    for g in range(NGRP):
                        nc.scalar.dma_start(
                            out=gout[by, g],
                            in_=ob[g * BB : (g + 1) * BB, :, g * UU : g * UU + GCOL],
                        )

    nc.compile()
    return nc


def _get_nc(reps=1):
    key = ("nc", reps)
    if key not in _CACHE:
        _CACHE[key] = _build_bass(reps)
    return _CACHE[key]


def _gather_index():
    """Flat indices into a [BB, GCOL] group tile: idx[j, dyp, dxp]."""
    if "idx" not in _CACHE:
        j = np.arange(BB)[:, None, None]
        dyp = np.arange(NG)[None, :, None]
        dxp = np.arange(NG)[None, None, :]
        col = dyp * UU + j + dxp
        _CACHE["idx"] = (j * GCOL + col).reshape(-1)
    return _CACHE["idx"]

def _extract(gout_arr):
    """[NBY, 8, BB, NBX, GCOL] fp16 group tiles -> [Q, H, W] fp32 output."""
    g2 = np.ascontiguousarray(
        np.asarray(gout_arr).transpose(0, 1, 3, 2, 4)
    )  # [by, g, bx, j, GCOL]
    flat = g2.reshape(NBY, NGRP, NBX, BB * GCOL)
    sub = flat[..., _gather_index()].astype(np.float32)
    sub = sub.reshape(NBY, NGRP, NBX, BB, NG, NG)
    # [by, g, bx, j, dyp, dxp] -> [dyp, dxp, (by,g), (bx,j)]
    return sub.transpose(4, 5, 0, 1, 2, 3).reshape(Q, H, W)


def prepare_inputs(data1, data2):
    """Full [N,C,H,W] fp32 inputs -> per-core in_maps (fp16, d1 pre-scaled)."""
    d1h = (np.asarray(data1, dtype=np.float32) * np.float32(1.0 / C)).astype(
        np.float16
    )
    d2h = np.zeros((N_CORES, C, H, WPX), dtype=np.float16)
    d2h[:, :, :, PAD : PAD + W] = np.asarray(data2, dtype=np.float16)
    # [N, C, H, W] -> m=(g,j) blocked, by-major: [N, 2, 128, NBY, NBX, 128]
    d1t = (
        d1h.reshape(N_CORES, C, NBY, GB, NBX, BB)
        .transpose(0, 1, 2, 4, 3, 5)
        .reshape(N_CORES, 2, 128, NBY, NBX, 128)
    )
    d2t = d2h.reshape(N_CORES, 2, 128, H, WPX)
    return [
        {
            "d1": np.ascontiguousarray(d1t[i]),
            "d2": np.ascontiguousarray(d2t[i]),
        }
        for i in range(N_CORES)
    ]


def _get_runner(reps=1):
    """Cached jit'd shard_map executable: f(*concat_inputs) -> concat outputs.

    Modeled on concourse.bass2jax.run_bass_via_pjrt, but built once and
    reusable so repeated kernel() calls (and timing loops) skip re-tracing.
    """
    rkey = ("runner", reps)
    if rkey in _CACHE:
        return _CACHE[rkey]

    import jax
    from jax.sharding import Mesh, PartitionSpec
    from jax.experimental.shard_map import shard_map
    import concourse.mybir as mybir
    from concourse import bass2jax

    bass2jax.install_neuronx_cc_hook()
    nc = _get_nc(reps)

    partition_name = nc.partition_id_tensor.name if nc.partition_id_tensor else None
    in_names, out_names, out_avals = [], [], []
    for alloc in nc.m.functions[0].allocations:
        if not isinstance(alloc, mybir.MemoryLocationSet):
            continue
        name = alloc.memorylocations[0].name
        if alloc.kind == "ExternalInput":
            if name != partition_name:
                in_names.append(name)
        elif alloc.kind == "ExternalOutput":
            out_names.append(name)
            out_avals.append(
                jax.core.ShapedArray(
                    tuple(alloc.tensor_shape), mybir.dt.np(alloc.dtype)
                )
            )
    n_params = len(in_names)
    all_in_names = in_names + out_names
    if partition_name is not None:
        all_in_names = all_in_names + [partition_name]

    def _body(*args):
        operands = list(args)
        if partition_name is not None:
            operands.append(bass2jax.partition_id_tensor())
        outs = bass2jax._bass_exec_p.bind(
            *operands,
            out_avals=tuple(out_avals),
            in_names=tuple(all_in_names),
            out_names=tuple(out_names),
            lowering_input_output_aliases=(),
            sim_require_finite=True,
            sim_require_nnan=True,
            nc=nc,
        )
        return tuple(outs)

    devices = jax.devices()[:N_CORES]
    mesh = Mesh(np.asarray(devices), ("core",))
    n_outs = len(out_names)
    sharded = jax.jit(
        shard_map(
            _body,
            mesh=mesh,
            in_specs=(PartitionSpec("core"),) * (n_params + n_outs),
            out_specs=(PartitionSpec("core"),) * n_outs,
            check_rep=False,
        ),
        keep_unused=True,
    )
    runner = {
        "fn": sharded,
        "in_names": in_names,
        "out_names": out_names,
        "out_avals": out_avals,
        "mesh": mesh,
    }
    _CACHE[rkey] = runner
    return runner


def run_hw(in_maps):
    """Execute on 8 cores; returns list of per-core {name: np.ndarray}."""
    r = _get_runner()
    concat_in = [
        np.concatenate([m[name] for m in in_maps], axis=0) for name in r["in_names"]
    ]
    concat_zeros = [
        np.zeros((N_CORES * a.shape[0], *a.shape[1:]), a.dtype)
        for a in r["out_avals"]
    ]
    out_arrs = r["fn"](*concat_in, *concat_zeros)
    return [
        {
            name: np.asarray(out_arrs[i]).reshape(
                N_CORES, *r["out_avals"][i].shape
            )[c]
            for i, name in enumerate(r["out_names"])
        }
        for c in range(N_CORES)
    ]


def kernel(data1, data2):
    in_maps = prepare_inputs(data1, data2)
    results = run_hw(in_maps)
    out = np.stack([_extract(r["gout"]) for r in results])
    return out.astype(np.float32)


# revision 6
# speedup vs baseline: 1.0587x; 1.0587x over previous
"""FlowNet-style correlation layer (MAX_DISPLACEMENT=4, 81 channels) on 8 TRN2 cores.

Strategy
--------
Data-parallel over batch N=8 -> 1 sample per NeuronCore.

Per core, for each 8x16 spatial block of data1 (the "stationary" block, M=128
positions) we matmul against the surrounding 16x24 patch of zero-padded data2
(the "moving" block, N=384 columns), contracting over C=256 in two K=128
chunks accumulated in PSUM.  The [128, 384] PSUM tile contains, for every
stationary position m=(g,j), all 81 correlation values at band columns
(g+dyp)*24 + (j+dxp).

DMA-volume optimizations over the naive banded scheme:
  * d2 is staged ONCE in SBUF as a persistent [128, 2, 104, 168] tile
    (x-padding baked on host, y-padding memset on-chip) -> no overlapping
    slab duplication.  d1 is staged raw [128, 2, 96, 160]; matmul operands
    are strided 2D-AP slices of the persistent tiles.
  * the band output is compacted per 16-partition group g to the 216 band
    columns [24g, 24g+216) that group actually uses (81/216 useful vs
    81/384) before leaving the chip.
  * d1 is pre-scaled by 1/C on the host so the PSUM->SBUF drain is a plain
    fp32->fp16 copy.

The final per-(g,j) diagonal gather (81 of 216 columns, per-partition offset)
happens on the host in numpy - per-partition-varying offsets are not
expressible in a single engine/DMA access pattern.
"""

import numpy as np

C, H, W = 256, 96, 160
PAD = 4
NG = 9  # displacement grid width (2*4+1)
Q = NG * NG  # 81 output channels
GB, BB = 8, 16  # stationary block: GB y-rows x BB x-cols = 128 positions
TT, UU = GB + 2 * PAD, BB + 2 * PAD  # moving block: 16 rows x 24 cols
NBY, NBX = H // GB, W // BB  # 12 x 10 = 120 blocks
NMOV = TT * UU  # 384 moving columns per matmul
HP, WPX = H + 2 * PAD, W + 2 * PAD  # padded dims (104, 168)
NGRP = 8  # partition groups of 16 (one per g)
GCOL = NG * UU  # 216 useful band columns per group
N_CORES = 8

_CACHE = {}


def _build_bass(reps=1):
    import contextlib

    import concourse.bass as bass  # noqa: F401
    import concourse.mybir as mybir
    import concourse.tile as tile
    from concourse import bacc

    fp16 = mybir.dt.float16
    fp32 = mybir.dt.float32

    nc = bacc.Bacc("TRN2", target_bir_lowering=False, debug=False)

    # d1: [2k, 128p, NBY, NBX, 128m] host-blocked m=(g,j) (pre-scaled by 1/C)
    # -- the matmul stationary AP must be a single free dim.  d2: [2k, 128p,
    # H, W+2*PAD] (x-padding baked in on host).
    d1 = nc.dram_tensor(
        "d1", [2, 128, NBY, NBX, 128], fp16, kind="ExternalInput"
    ).ap()
    d2 = nc.dram_tensor("d2", [2, 128, H, WPX], fp16, kind="ExternalInput").ap()
    gout = nc.dram_tensor(
        "gout", [NBY, NGRP, BB, NBX, GCOL], fp16, kind="ExternalOutput"
    ).ap()

    d1r = d1.rearrange("k p by bx m -> p k by bx m")
    d2r = d2.rearrange("k p h w -> p k h w")

    with tile.TileContext(nc) as tc:
        with (
            tc.tile_pool(name="in1", bufs=1) as in1_pool,
            tc.tile_pool(name="in2", bufs=1) as in2_pool,
            tc.tile_pool(name="ps", bufs=8, space="PSUM") as ps_pool,
            tc.tile_pool(name="ob", bufs=3) as ob_pool,
        ):
            loop = tc.For_i(0, reps, 1) if reps > 1 else contextlib.nullcontext()
            with loop:
                d1_sb = in1_pool.tile([128, 2, NBY, NBX, 128], fp16, tag="d1sb")
                d2_sb = in2_pool.tile([128, 2, HP, WPX], fp16, tag="d2sb")
                # y-pad rows of the persistent d2 tile.
                nc.vector.memset(d2_sb[:, :, 0:PAD, :], 0.0)
                nc.vector.memset(d2_sb[:, :, H + PAD : HP, :], 0.0)
                # Chunked loads (8 y-rows each) so compute starts early;
                # separate queues so the two input streams overlap.
                for by in range(NBY):
                    y0 = by * GB
                    nc.gpsimd.dma_start(
                        out=d1_sb[:, :, by], in_=d1r[:, :, by]
                    )
                    nc.sync.dma_start(
                        out=d2_sb[:, :, PAD + y0 : PAD + y0 + GB, :],
                        in_=d2r[:, :, y0 : y0 + GB, :],
                    )
                for by in range(NBY):
                    y0 = by * GB
                    ob = ob_pool.tile([128, NBX, NMOV], fp16, tag="ob")
                    for bx in range(NBX):
                        x0 = bx * BB
                        ps = ps_pool.tile([128, NMOV], fp32)
                        for k in range(2):
                            lhsT = d1_sb[:, k, by, bx, :]
                            rhs = d2_sb[:, k, y0 : y0 + TT, x0 : x0 + UU]
                            nc.tensor.matmul(
                                ps[:], lhsT, rhs, start=(k == 0), stop=(k == 1)
                            )
                        if bx % 2 == 1:
                            nc.scalar.copy(ob[:, bx, :], ps[:])
                        else:
                            nc.vector.tensor_copy(ob[:, bx, :], ps[:])
                    # Grouped band compaction: partition group g only uses
                    # band columns [24g, 24g+216).  A 16-partition DMA only
                    # engages 4 of 16 SBUF AXI ports, so spread the 8 group
                    # DMAs across all three DMA queues (adjacent groups hit
                    # complementary port sets) to keep them concurrent.
                    for g in range(NGRP):
                        eng = (nc.scalar, nc.sync, nc.gpsimd)[g % 3]
                        eng.dma_start(
                            out=gout[by, g],
                            in_=ob[g * BB : (g + 1) * BB, :, g * UU : g * UU + GCOL],
                        )

    nc.compile()
    return nc


def _get_nc(reps=1):
    key = ("nc", reps)
    if key not in _CACHE:
        _CACHE[key] = _build_bass(reps)
    return _CACHE[key]


def _gather_index():
    """Flat indices into a [BB, GCOL] group tile: idx[j, dyp, dxp]."""
    if "idx" not in _CACHE:
        j = np.arange(BB)[:, None, None]
        dyp = np.arange(NG)[None, :, None]
        dxp = np.arange(NG)[None, None, :]
        col = dyp * UU + j + dxp
        _CACHE["idx"] = (j * GCOL + col).reshape(-1)
    return _CACHE["idx"]

def _extract(gout_arr):
    """[NBY, 8, BB, NBX, GCOL] fp16 group tiles -> [Q, H, W] fp32 output."""
    g2 = np.ascontiguousarray(
        np.asarray(gout_arr).transpose(0, 1, 3, 2, 4)
    )  # [by, g, bx, j, GCOL]
    flat = g2.reshape(NBY, NGRP, NBX, BB * GCOL)
    sub = flat[..., _gather_index()].astype(np.float32)
    sub = sub.reshape(NBY, NGRP, NBX, BB, NG, NG)
    # [by, g, bx, j, dyp, dxp] -> [dyp, dxp, (by,g), (bx,j)]
    return sub.transpose(4, 5, 0, 1, 2, 3).reshape(Q, H, W)


def prepare_inputs(data1, data2):
    """Full [N,C,H,W] fp32 inputs -> per-core in_maps (fp16, d1 pre-scaled)."""
    d1h = (np.asarray(data1, dtype=np.float32) * np.float32(1.0 / C)).astype(
        np.float16
    )
    d2h = np.zeros((N_CORES, C, H, WPX), dtype=np.float16)
    d2h[:, :, :, PAD : PAD + W] = np.asarray(data2, dtype=np.float16)
    # [N, C, H, W] -> m=(g,j) blocked, by-major: [N, 2, 128, NBY, NBX, 128]
    d1t = (
        d1h.reshape(N_CORES, C, NBY, GB, NBX, BB)
        .transpose(0, 1, 2, 4, 3, 5)
        .reshape(N_CORES, 2, 128, NBY, NBX, 128)
    )
    d2t = d2h.reshape(N_CORES, 2, 128, H, WPX)
    return [
        {
            "d1": np.ascontiguousarray(d1t[i]),
            "d2": np.ascontiguousarray(d2t[i]),
        }
        for i in range(N_CORES)
    ]


def _get_runner(reps=1):
    """Cached jit'd shard_map executable: f(*concat_inputs) -> concat outputs.

    Modeled on concourse.bass2jax.run_bass_via_pjrt, but built once and
    reusable so repeated kernel() calls (and timing loops) skip re-tracing.
    """
    rkey = ("runner", reps)
    if rkey in _CACHE:
        return _CACHE[rkey]

    import jax
    from jax.sharding import Mesh, PartitionSpec
    from jax.experimental.shard_map import shard_map
    import concourse.mybir as mybir
    from concourse import bass2jax

    bass2jax.install_neuronx_cc_hook()
    nc = _get_nc(reps)

    partition_name = nc.partition_id_tensor.name if nc.partition_id_tensor else None
    in_names, out_names, out_avals = [], [], []
    for alloc in nc.m.functions[0].allocations:
        if not isinstance(alloc, mybir.MemoryLocationSet):
            continue
        name = alloc.memorylocations[0].name
        if alloc.kind == "ExternalInput":
            if name != partition_name:
                in_names.append(name)
        elif alloc.kind == "ExternalOutput":
            out_names.append(name)
            out_avals.append(
                jax.core.ShapedArray(
                    tuple(alloc.tensor_shape), mybir.dt.np(alloc.dtype)
                )
            )
    n_params = len(in_names)
    all_in_names = in_names + out_names
    if partition_name is not None:
        all_in_names = all_in_names + [partition_name]

    def _body(*args):
        operands = list(args)
        if partition_name is not None:
            operands.append(bass2jax.partition_id_tensor())
        outs = bass2jax._bass_exec_p.bind(
            *operands,
            out_avals=tuple(out_avals),
            in_names=tuple(all_in_names),
            out_names=tuple(out_names),
            lowering_input_output_aliases=(),
            sim_require_finite=True,
            sim_require_nnan=True,
            nc=nc,
        )
        return tuple(outs)

    devices = jax.devices()[:N_CORES]
    mesh = Mesh(np.asarray(devices), ("core",))
    n_outs = len(out_names)
    sharded = jax.jit(
        shard_map(
            _body,
            mesh=mesh,
            in_specs=(PartitionSpec("core"),) * (n_params + n_outs),
            out_specs=(PartitionSpec("core"),) * n_outs,
            check_rep=False,
        ),
        keep_unused=True,
    )
    runner = {
        "fn": sharded,
        "in_names": in_names,
        "out_names": out_names,
        "out_avals": out_avals,
        "mesh": mesh,
    }
    _CACHE[rkey] = runner
    return runner


def run_hw(in_maps):
    """Execute on 8 cores; returns list of per-core {name: np.ndarray}."""
    r = _get_runner()
    concat_in = [
        np.concatenate([m[name] for m in in_maps], axis=0) for name in r["in_names"]
    ]
    concat_zeros = [
        np.zeros((N_CORES * a.shape[0], *a.shape[1:]), a.dtype)
        for a in r["out_avals"]
    ]
    out_arrs = r["fn"](*concat_in, *concat_zeros)
    return [
        {
            name: np.asarray(out_arrs[i]).reshape(
                N_CORES, *r["out_avals"][i].shape
            )[c]
            for i, name in enumerate(r["out_names"])
        }
        for c in range(N_CORES)
    ]


def kernel(data1, data2):
    in_maps = prepare_inputs(data1, data2)
    results = run_hw(in_maps)
    out = np.stack([_extract(r["gout"]) for r in results])
    return out.astype(np.float32)
